# revision 5
# baseline (speedup 1.0000x reference)
"""Bass/Trainium2 kernel for nn_CopyGenerator (8-core SPMD).

Sharding: 4-way vocab (tensor parallel) x 2-way rows (data parallel).
Core c = 4*h + q owns rows [2048h, 2048h+2048) and vocab columns
[8000q, 8000q+8000).  The softmax denominator needs a cross-vocab-shard
sum: one AllReduce over 4 ranks per tapered group of row-blocks (GS),
in two independent replica groups ([[0,1,2,3],[4,5,6,7]]) that pipeline
behind compute.  The copy branch stays batch-sharded 8 ways (4
batches/core).  A tiny warmup NEFF with one AllReduce runs first: the
first collective after device boot pays ~60-75us of channel start
latency that would otherwise stall the main kernel.

The big matmul runs in fp8 e4m3 with perf_mode=DoubleRow (2 fp8
weights/PE cell, K=256 per pass -> ~2x bf16 FLOP rate).  hidden is
scaled x16 and W x64 before the e4m3 cast so the bulk of both
distributions sits in the normal range (min normal 2^-6); the Exp
activation un-scales via its fp32 `scale` operand (1/1024).  fp8
quantization adds ~3-4% relative noise to individual softmax probs,
which is far inside the 2e-2 budget because gen-branch probs (~2e-4)
are tiny against the copy-branch absmax (~0.1) and the denominator
noise averages out over 32000 terms.

The copy-gate logit is FOLDED into the big matmul as one extra W
column (col V_loc of the padded shard): sigmoid(x) = ep/(1+ep) with
ep = exp(x) falling out of the same Exp pass, so the per-block scale
is m = (1-gate)/S = 1/((1+ep*e^{b_copy}) * (S_allreduce - pad_corr)).
This removes 64 N=1 gate matmuls (each paying a full LDWEIGHTS) from
the PE stream.

Per 128-row block:
  - PE: logits into PSUM (2 DoubleRow K-passes x <=512-col matmuls).
  - ACT: Exp (scale=1/1024), exp values kept in SBUF fp16.  The
    free-dim partial sums are split between ACT accum_out (5 chunks)
    and DVE reduce (3 chunks): either engine alone would be
    co-critical with PE once the matmul runs at fp8 speed
    (ACT 980ns/exp + 284ns/accum-readout; DVE reduce 1070ns/chunk).
  - After the group all-reduce: DVE scales exp by (1-gate)/S and the
    result is stored as bf16 (host upcasts; probs are ~1e-4 so bf16
    rounding is ~1e-7 absolute).
  - PAD masking: host zeroes W[PAD,:] (dead data in the reference), the
    resulting constant exp(0)=1 is subtracted from the reduced sum, and
    the host zeroes output column PAD.
  - Copy branch: fp16 matmul (one-hot src_map is exact in fp16); its
    gate ALSO needs ~1e-3 accuracy (it multiplies values that ARE the
    output absmax) so it gets its own fp16 dot product -- fp16 is
    ~5e-4 there, e4m3 would be ~2-4%.
"""

import os
import sys

for _p in ("/opt/trn_rl_repo", "/root/.axon_site/_ro/trn_rl_repo"):
    if os.path.isdir(_p) and _p not in sys.path:
        sys.path.insert(0, _p)

import numpy as np
import ml_dtypes

import concourse.bacc as bacc
import concourse.tile as tile
from concourse import mybir
from concourse.bass_utils import run_bass_kernel_spmd

# ---------------------------------------------------------------------------
# Problem dimensions (hardcoded per spec)
# ---------------------------------------------------------------------------
B, T, S, V, CV, D = 32, 128, 400, 32000, 600, 512
PAD = 1
NCORES = 8
NQ = 4                    # vocab shards
NH = 2                    # row halves
R = B * T                 # 4096 rows
VS = V // NQ              # 8000 vocab columns per core
VSP = VS + 16             # padded shard: col VS = w_copy, cols VS+1.. = 0
RH = R // NH              # 2048 rows per core
RB = 128                  # rows per block (= one batch: T == 128)
NBL = RH // RB            # 16 row blocks per core
# tapered all-reduce groups: small first group fills the pipeline before the
# exp pool saturates; tiny last groups shrink the drain tail
GS = [2, 3, 3, 3, 2, 2, 1]   # sums to NBL
NG = len(GS)
GOFF = [sum(GS[:i]) for i in range(NG)]
GRPOF = []                # block -> (group, index-in-group)
for _g, _n in enumerate(GS):
    for _j in range(_n):
        GRPOF.append((_g, _j))
LB = B // NCORES          # 4 local batches per core (copy branch)
KC = D // 128             # 4 contraction chunks (2 DoubleRow passes)
# vocab chunking within a block (PSUM: [128,1024]f32 = 2 banks).
# Chunk 7 is 848 wide: 832 real vocab cols + gate col (832) + 15 zero pads.
VCH = [1024] * 7 + [848]  # matmul/psum width
RW7 = 832                 # chunk-7 softmax width (excludes gate + pads)
VOFF = [1024 * i for i in range(8)]
NVC = len(VCH)
ACT_ACC = (0, 1, 2, 3, 7)  # chunks whose partial sum runs on ACT accum_out
# s-dim chunks for the copy branch: 400 = 128+128+128+16
SCH = [128, 128, 128, 16]
SOFF = [0, 128, 256, 384]

F32 = mybir.dt.float32
F16 = mybir.dt.float16
BF16 = mybir.dt.bfloat16
F8 = mybir.dt.float8e4
DR = mybir.MatmulPerfMode.DoubleRow

# fp8 pre-scales (host multiplies before the e4m3 cast; Exp un-scales)
SH = 16.0                 # hidden scale
SW = 64.0                 # W / w_copy scale
INV = 1.0 / (SH * SW)     # 1/1024

EXP_BUFS = 48   # in-flight exp tiles ([128,1024] f16)
OUT_BUFS = 2    # [128, 4096] bf16 output staging tiles (2 per block)


def _mm_splits(n):
    """Split a free-dim span into <=512 pieces aligned to 512 (PSUM banks)."""
    out = []
    off = 0
    while off < n:
        w = min(512, n - off)
        out.append((off, w))
        off += w
    return out


def build_program(with_bias: bool, b_copy: float, pad_corr: float):
    # Bacc (not plain Bass): its finalize() runs move_matmul_waits_to_ldweights
    # + generate_event_semaphores, which split multi-sem waits down to the
    # TRN2 limit of one wait per instruction — walrus rejects the IR otherwise.
    nc = bacc.Bacc()

    ebc = float(np.exp(b_copy))

    h8d = nc.dram_tensor("h8", [D, RH], F8, kind="ExternalInput")
    w8d = nc.dram_tensor("w8", [D, VSP], F8, kind="ExternalInput")
    h16d = nc.dram_tensor("h16", [D, LB * RB], F16, kind="ExternalInput")
    wc16d = nc.dram_tensor("wc16", [D, 1], F16, kind="ExternalInput")
    attnT = nc.dram_tensor("attnT", [S, LB * RB], F16, kind="ExternalInput")
    smap = nc.dram_tensor("smap", [LB, S, CV], F16, kind="ExternalInput")
    if with_bias:
        ebb = nc.dram_tensor("ebb", [128, VS], F32, kind="ExternalInput")

    og = nc.dram_tensor("og", [RH, VS], BF16, kind="ExternalOutput")
    oc = nc.dram_tensor("oc", [LB * RB, CV], F32, kind="ExternalOutput")

    with tile.TileContext(nc) as tc:
        with (
            tc.tile_pool(name="const", bufs=1) as const,
            tc.tile_pool(name="pm", bufs=3, space="PSUM") as pm,
            tc.tile_pool(name="pg", bufs=2, space="PSUM") as pg,
            tc.tile_pool(name="expp", bufs=EXP_BUFS) as expp,
            tc.tile_pool(name="outp", bufs=OUT_BUFS) as outp,
            tc.tile_pool(name="ocp", bufs=2) as ocp,
            tc.tile_pool(name="smapp", bufs=4) as smapp,
            tc.tile_pool(name="small", bufs=10) as small,
            tc.tile_pool(name="gatep", bufs=NBL + LB) as gatep,
            tc.tile_pool(name="dram", bufs=1, space="DRAM") as dram,
        ):
            # ---------------- prologue: resident loads ----------------
            # fp8 residents as [128, KC, free] k-plane tiles (the middle dim
            # is the DoubleRow k-pair axis).  hidden on the gpsimd ring; the
            # 4 MB W shard in 2048-col slices split across the ACT and DVE
            # rings so block 0's chunks land within ~2-3us.
            h8t = const.tile([128, KC, RH], F8, tag="h8t", name="h8t")
            for k in range(KC):
                nc.gpsimd.dma_start(h8t[:, k:k + 1, :], h8d[k * 128:(k + 1) * 128, :])
            w8t = const.tile([128, KC, VSP], F8, tag="w8t", name="w8t")
            w_slices = [(0, 2048), (2048, 4096), (4096, 6144), (6144, VSP)]
            for (vo, ve) in w_slices:
                for k in range(KC):
                    # DMA-capable queues are SP/ACT/gpsimd only; split the
                    # 4 MB shard across the ACT and SP rings
                    eng = nc.scalar if k < 2 else nc.sync
                    eng.dma_start(
                        w8t[:, k:k + 1, vo:ve],
                        w8d[k * 128:(k + 1) * 128, vo:ve],
                    )
            # copy-branch inputs on the SP ring
            h16_t = []
            wc_t = []
            attnT_t = []
            ebb_t = []
            for k in range(KC):
                th = const.tile([128, LB * RB], F16, tag=f"h16_{k}", name=f"h16_{k}")
                nc.sync.dma_start(th[:], h16d[k * 128:(k + 1) * 128, :])
                h16_t.append(th)
                tw = const.tile([128, 1], F16, tag=f"wc16_{k}", name=f"wc16_{k}")
                nc.sync.dma_start(tw[:], wc16d[k * 128:(k + 1) * 128, :])
                wc_t.append(tw)
            for k in range(4):
                sk = SCH[k]
                t = const.tile([128, LB * RB], F16, tag=f"attnT{k}", name=f"attnT{k}")
                nc.sync.dma_start(t[:sk, :], attnT[SOFF[k]:SOFF[k] + sk, :])
                attnT_t.append(t)
            if with_bias:
                for i in range(NVC):
                    n = VCH[i] if i < 7 else RW7
                    t = const.tile([128, n], F32, tag=f"ebb{i}", name=f"ebb{i}")
                    nc.sync.dma_start(t[:], ebb[:, VOFF[i]:VOFF[i] + n])
                    ebb_t.append(t)

            # ---------------- main loop ----------------
            exp_tiles = [[None] * NVC for _ in range(NBL)]
            sg_tiles = [None] * NG    # group local sums [128, GROUP]
            cc_out = [None] * NG      # group all-reduced sums (SBUF)

            def compute_block(jb):
                cb = slice(jb * RB, (jb + 1) * RB)
                sp = small.tile([128, NVC], F32, tag="sp", name="sp")
                for i in range(NVC):
                    n = VCH[i]
                    rw = n if i < 7 else RW7
                    ps = pm.tile([128, 1024], F32, tag="pm", name="pm")
                    for kk in range(KC // 2):
                        for (o, w) in _mm_splits(n):
                            nc.tensor.matmul(
                                ps[:, o:o + w],
                                h8t[:, 2 * kk:2 * kk + 2, cb],
                                w8t[:, 2 * kk:2 * kk + 2,
                                    VOFF[i] + o:VOFF[i] + o + w],
                                start=(kk == 0), stop=(kk == KC // 2 - 1),
                                perf_mode=DR,
                            )
                    ex = expp.tile([128, 1024], F16, tag="exp", name="exp")
                    if with_bias:
                        nc.scalar.activation(
                            ex[:, :rw], ps[:, :rw],
                            mybir.ActivationFunctionType.Exp, scale=INV,
                        )
                        nc.vector.tensor_tensor(
                            ex[:, :rw], ex[:, :rw], ebb_t[i][:, :rw],
                            mybir.AluOpType.mult,
                        )
                        nc.vector.reduce_sum(
                            sp[:, i:i + 1], ex[:, :rw],
                            axis=mybir.AxisListType.X,
                        )
                    elif i in ACT_ACC:
                        nc.scalar.activation(
                            ex[:, :rw], ps[:, :rw],
                            mybir.ActivationFunctionType.Exp, scale=INV,
                            accum_out=sp[:, i:i + 1],
                        )
                    else:
                        nc.scalar.activation(
                            ex[:, :rw], ps[:, :rw],
                            mybir.ActivationFunctionType.Exp, scale=INV,
                        )
                        nc.vector.reduce_sum(
                            sp[:, i:i + 1], ex[:, :rw],
                            axis=mybir.AxisListType.X,
                        )
                    if i == 7:
                        # folded copy-gate numerator ep = exp(h . w_copy)
                        nc.scalar.activation(
                            ex[:, RW7:RW7 + 1], ps[:, RW7:RW7 + 1],
                            mybir.ActivationFunctionType.Exp, scale=INV,
                        )
                    exp_tiles[jb][i] = ex
                g, j = GRPOF[jb]
                nc.vector.reduce_sum(
                    sg_tiles[g][:, j:j + 1], sp[:], axis=mybir.AxisListType.X
                )

            def scale_block(jb):
                g, j = GRPOF[jb]
                sgl = cc_out[g]
                ept = exp_tiles[jb][7][:, RW7:RW7 + 1]
                # m = (1-gate)/S = 1 / ((1 + ep*e^{b_copy}) * (S - pad_corr))
                upl = small.tile([128, 1], F32, tag="upl", name="upl")
                if ebc == 1.0:
                    nc.vector.tensor_scalar_add(upl[:], ept, 1.0)
                else:
                    nc.vector.tensor_scalar(
                        upl[:], ept, ebc, 1.0,
                        mybir.AluOpType.mult, mybir.AluOpType.add,
                    )
                corr = small.tile([128, 1], F32, tag="corr", name="corr")
                nc.vector.tensor_scalar_add(corr[:], sgl[:, j:j + 1], -pad_corr)
                v = small.tile([128, 1], F32, tag="v", name="v")
                nc.vector.tensor_scalar(
                    v[:], corr[:], upl[:], None, mybir.AluOpType.mult
                )
                m = small.tile([128, 1], F32, tag="m", name="m")
                nc.vector.reciprocal(m[:], v[:])
                # scale exp chunks into bf16 staging tiles, 2 stores per block
                for half in range(2):
                    hn = 4096 if half == 0 else VS - 4096
                    ot = outp.tile([128, 4096], BF16, tag="ot", name="ot")
                    for i in range(4 * half, 4 * half + 4):
                        n = VCH[i] if i < 7 else RW7
                        oo = VOFF[i] - 4096 * half
                        nc.vector.tensor_scalar(
                            ot[:, oo:oo + n],
                            exp_tiles[jb][i][:, :n], m[:], None,
                            mybir.AluOpType.mult,
                        )
                        exp_tiles[jb][i] = None
                    nc.sync.dma_start(
                        og[jb * RB:(jb + 1) * RB, 4096 * half:4096 * half + hn],
                        ot[:, :hn],
                    )

            # ---------------- copy branch (batch-sharded) ----------------
            def emit_copy_branch():
                for l in range(LB):
                    tb = slice(l * RB, (l + 1) * RB)
                    # local gate: fp16 dot (sigma ~5e-4 -- the gate multiplies
                    # values that set the output absmax, so fp8 is too coarse)
                    gps = pg.tile([128, 1], F32, tag="gate", name="gate")
                    for k in range(KC):
                        nc.tensor.matmul(
                            gps[:], h16_t[k][:, tb], wc_t[k][:],
                            start=(k == 0), stop=(k == KC - 1),
                        )
                    el = gatep.tile([128, 1], F32, tag="el", name="el")
                    nc.scalar.activation(
                        el[:], gps[:], mybir.ActivationFunctionType.Exp,
                        bias=-float(b_copy), scale=-1.0,
                    )
                    ul = gatep.tile([128, 1], F32, tag="ul", name="ul")
                    nc.vector.tensor_scalar_add(ul[:], el[:], 1.0)
                    gl = gatep.tile([128, 1], F32, tag="gl", name="gl")
                    nc.vector.reciprocal(gl[:], ul[:])
                    cps = pm.tile([128, 1024], F32, tag="pm", name="cp")
                    for k in range(4):
                        sk = SCH[k]
                        sm = smapp.tile([128, CV], F16, tag="sm", name="sm")
                        nc.gpsimd.dma_start(
                            sm[:sk, :], smap[l, SOFF[k]:SOFF[k] + sk, :]
                        )
                        for (o, w) in _mm_splits(CV):
                            nc.tensor.matmul(
                                cps[:, o:o + w],
                                attnT_t[k][:sk, tb],
                                sm[:sk, o:o + w],
                                start=(k == 0), stop=(k == 3),
                            )
                    oct_ = ocp.tile([128, CV], F32, tag="oct", name="oct")
                    nc.vector.tensor_scalar(
                        oct_[:], cps[:, :CV], gl[:], None, mybir.AluOpType.mult
                    )
                    nc.sync.dma_start(oc[tb, :], oct_[:])

            for g in range(NG):
                gn = GS[g]
                sg_tiles[g] = small.tile([128, gn], F32, tag="sg", name="sg")
                for j in range(gn):
                    compute_block(GOFF[g] + j)
                # all-reduce this group's local sums across the 4 vocab shards
                cin = dram.tile([128, gn], F32, tag=f"cin{g}", name=f"cin{g}")
                cout = dram.tile([128, gn], F32, tag=f"cout{g}", name=f"cout{g}")
                nc.gpsimd.dma_start(cin[:], sg_tiles[g][:])
                nc.gpsimd.collective_compute(
                    "AllReduce",
                    mybir.AluOpType.add,
                    replica_groups=[[0, 1, 2, 3], [4, 5, 6, 7]],
                    ins=[cin.opt()],
                    outs=[cout.opt()],
                )
                sgl = small.tile([128, gn], F32, tag="sgl", name="sgl")
                nc.gpsimd.dma_start(sgl[:], cout[:])
                cc_out[g] = sgl
                if g == 0:
                    # PE-only work that needs neither w8 nor any collective:
                    # fills the PE while group 0's all-reduce is in flight
                    emit_copy_branch()
                for j in range(gn):
                    scale_block(GOFF[g] + j)


    nc.finalize()
    return nc


_warmed_up = False


def _warmup_collectives():
    """Run a minimal NEFF with one AllReduce so the collective channel
    (ncfw firmware / TOPSP) is warm before the main kernel executes —
    the first collective after boot costs ~60-75us of start latency."""
    global _warmed_up
    if _warmed_up:
        return
    nc = bacc.Bacc()
    x = nc.dram_tensor("x", [128, 4], F32, kind="ExternalInput")
    y = nc.dram_tensor("y", [128, 4], F32, kind="ExternalOutput")
    with tile.TileContext(nc) as tc:
        with (
            tc.tile_pool(name="sb", bufs=2) as sb,
            tc.tile_pool(name="dr", bufs=2, space="DRAM") as dr,
        ):
            t = sb.tile([128, 4], F32, tag="t", name="t")
            nc.sync.dma_start(t[:], x[:])
            bi = dr.tile([128, 4], F32, tag="bi", name="bi")
            bo = dr.tile([128, 4], F32, tag="bo", name="bo")
            nc.sync.dma_start(bi[:], t[:])
            nc.gpsimd.collective_compute(
                "AllReduce",
                mybir.AluOpType.add,
                replica_groups=[[0, 1, 2, 3], [4, 5, 6, 7]],
                ins=[bi.opt()],
                outs=[bo.opt()],
            )
            t2 = sb.tile([128, 4], F32, tag="t2", name="t2")
            nc.sync.dma_start(t2[:], bo[:])
            nc.sync.dma_start(y[:], t2[:])
    nc.finalize()
    z = np.zeros((128, 4), np.float32)
    run_bass_kernel_spmd(nc, [{"x": z}] * NCORES, core_ids=list(range(NCORES)))
    _warmed_up = True


def kernel(hidden, copy_attn, src_map, W, b, w_copy, b_copy, _trace=False):
    hidden = np.asarray(hidden, np.float32)
    copy_attn = np.asarray(copy_attn, np.float32)
    src_map = np.asarray(src_map, np.float32)
    W = np.asarray(W, np.float32)
    b = np.asarray(b, np.float32)
    w_copy = np.asarray(w_copy, np.float32)
    b_copy_f = float(np.asarray(b_copy))

    with_bias = bool(np.any(b != 0.0))
    pad_corr = float(np.exp(b[PAD])) if with_bias else 1.0

    # host-side shard prep (layout only; W[PAD,:] is dead data in the ref)
    Wz = W.copy()
    Wz[PAD, :] = 0.0
    W8 = (np.ascontiguousarray(Wz.T) * SW).astype(ml_dtypes.float8_e4m3)  # [D, V]
    wc8 = (w_copy.reshape(D, 1) * SW).astype(ml_dtypes.float8_e4m3)
    hT_f = np.ascontiguousarray(hidden.T)                            # [D, R] f32
    h8 = (hT_f * SH).astype(ml_dtypes.float8_e4m3)
    h16 = hT_f.astype(np.float16)
    wc16 = w_copy.reshape(D, 1).astype(np.float16)
    attnT_full = np.ascontiguousarray(copy_attn.T).astype(np.float16)  # [S, R]
    smap16 = src_map.astype(np.float16)                              # [B,S,CV]

    _warmup_collectives()
    nc = build_program(with_bias, b_copy_f, pad_corr)

    in_maps = []
    zpad = np.zeros((D, 15), dtype=ml_dtypes.float8_e4m3)
    for c in range(NCORES):
        h, q = divmod(c, NQ)
        rows = slice(h * RH, (h + 1) * RH)
        crows = slice(c * LB * RB, (c + 1) * LB * RB)
        m = {
            "h8": np.ascontiguousarray(h8[:, rows]),
            "w8": np.ascontiguousarray(
                np.concatenate(
                    [W8[:, q * VS:(q + 1) * VS], wc8, zpad], axis=1
                )
            ),
            "h16": np.ascontiguousarray(h16[:, crows]),
            "wc16": wc16,
            "attnT": np.ascontiguousarray(attnT_full[:, crows]),
            "smap": np.ascontiguousarray(smap16[c * LB:(c + 1) * LB]),
        }
        if with_bias:
            eb = np.exp(b[q * VS:(q + 1) * VS].astype(np.float64)).astype(
                np.float32
            )
            m["ebb"] = np.ascontiguousarray(
                np.broadcast_to(eb[None, :], (128, VS))
            )
        in_maps.append(m)

    trace_cores = None
    if os.environ.get("TRACE_ALL_CORES"):
        trace_cores = list(range(NCORES))
    res = run_bass_kernel_spmd(
        nc, in_maps, core_ids=list(range(NCORES)), trace=_trace,
        trace_cores=trace_cores,
    )

    out = np.empty((R, V + CV), np.float32)
    for c in range(NCORES):
        h, q = divmod(c, NQ)
        out[h * RH:(h + 1) * RH, q * VS:(q + 1) * VS] = (
            res.results[c]["og"].astype(np.float32)
        )
        out[c * LB * RB:(c + 1) * LB * RB, V:] = res.results[c]["oc"]
    out[:, PAD] = 0.0

    if _trace:
        kernel.last_results = res
    return out


kernel.last_results = None


# revision 11
# speedup vs baseline: 1.0510x; 1.0510x over previous
"""Bass/Trainium2 kernel for nn_CopyGenerator (8-core SPMD).

Sharding: 4-way vocab (tensor parallel) x 2-way rows (data parallel).
Core c = 4*h + q owns rows [2048h, 2048h+2048) and vocab columns
[8000q, 8000q+8000).  The softmax denominator needs a cross-vocab-shard
sum: one AllReduce over 4 ranks per tapered group of row-blocks (GS),
in two independent replica groups ([[0,1,2,3],[4,5,6,7]]) that pipeline
behind compute.  The copy branch stays batch-sharded 8 ways (4
batches/core).  A tiny warmup NEFF with one AllReduce runs first: the
first collective after device boot pays ~60-75us of channel start
latency that would otherwise stall the main kernel.

The big matmul runs in fp8 e4m3 with perf_mode=DoubleRow (2 fp8
weights/PE cell, K=256 per pass -> ~2x bf16 FLOP rate).  hidden is
scaled x16 and W x64 before the e4m3 cast so the bulk of both
distributions sits in the normal range (min normal 2^-6); the Exp
activation un-scales via its fp32 `scale` operand (1/1024).  fp8
quantization adds ~3-4% relative noise to individual softmax probs,
which is far inside the 2e-2 budget because gen-branch probs (~2e-4)
are tiny against the copy-branch absmax (~0.1) and the denominator
noise averages out over 32000 terms.

The copy-gate logit is FOLDED into the big matmul as one extra W
column (col V_loc of the padded shard): sigmoid(x) = ep/(1+ep) with
ep = exp(x) falling out of the same Exp pass, so the per-block scale
is m = (1-gate)/S = 1/((1+ep*e^{b_copy}) * (S_allreduce - pad_corr)).
This removes 64 N=1 gate matmuls (each paying a full LDWEIGHTS) from
the PE stream.

Per 128-row block:
  - PE: logits into PSUM (2 DoubleRow K-passes x <=512-col matmuls).
  - ACT: Exp (scale=1/1024), exp values kept in SBUF fp16.  The
    free-dim partial sums are split between ACT accum_out (5 chunks)
    and DVE reduce (3 chunks): either engine alone would be
    co-critical with PE once the matmul runs at fp8 speed
    (ACT 980ns/exp + 284ns/accum-readout; DVE reduce 1070ns/chunk).
  - After the group all-reduce: DVE scales exp by (1-gate)/S and the
    result is stored as bf16 (host upcasts; probs are ~1e-4 so bf16
    rounding is ~1e-7 absolute).
  - PAD masking: host zeroes W[PAD,:] (dead data in the reference), the
    resulting constant exp(0)=1 is subtracted from the reduced sum, and
    the host zeroes output column PAD.
  - Copy branch: fp16 matmul (one-hot src_map is exact in fp16); its
    gate ALSO needs ~1e-3 accuracy (it multiplies values that ARE the
    output absmax) so it gets its own fp16 dot product -- fp16 is
    ~5e-4 there, e4m3 would be ~2-4%.
"""

import os
import sys

for _p in ("/opt/trn_rl_repo", "/root/.axon_site/_ro/trn_rl_repo"):
    if os.path.isdir(_p) and _p not in sys.path:
        sys.path.insert(0, _p)

import numpy as np
import ml_dtypes

import concourse.bacc as bacc
import concourse.tile as tile
from concourse import mybir
from concourse.bass_utils import run_bass_kernel_spmd

# ---------------------------------------------------------------------------
# Problem dimensions (hardcoded per spec)
# ---------------------------------------------------------------------------
B, T, S, V, CV, D = 32, 128, 400, 32000, 600, 512
PAD = 1
NCORES = 8
NQ = 4                    # vocab shards
NH = 2                    # row halves
R = B * T                 # 4096 rows
VS = V // NQ              # 8000 vocab columns per core
VSP = VS + 16             # padded shard: col VS = w_copy, cols VS+1.. = 0
RH = R // NH              # 2048 rows per core
RB = 128                  # rows per block (= one batch: T == 128)
NBL = RH // RB            # 16 row blocks per core
# tapered all-reduce groups: small first group fills the pipeline before the
# exp pool saturates; tiny last groups shrink the drain tail
GS = [2, 3, 3, 3, 2, 2, 1]   # sums to NBL
NG = len(GS)
GOFF = [sum(GS[:i]) for i in range(NG)]
GRPOF = []                # block -> (group, index-in-group)
for _g, _n in enumerate(GS):
    for _j in range(_n):
        GRPOF.append((_g, _j))
LB = B // NCORES          # 4 local batches per core (copy branch)
KC = D // 128             # 4 contraction chunks (2 DoubleRow passes)
# vocab chunking within a block (PSUM: [128,1024]f32 = 2 banks).
# Chunk 7 is 848 wide: 832 real vocab cols + gate col (832) + 15 zero pads.
VCH = [1024] * 7 + [848]  # matmul/psum width
RW7 = 832                 # chunk-7 softmax width (excludes gate + pads)
VOFF = [1024 * i for i in range(8)]
NVC = len(VCH)
ACT_ACC = (0, 1, 3, 7)    # chunks whose partial sum runs on ACT accum_out
# s-dim chunks for the copy branch: 400 = 128+128+128+16
SCH = [128, 128, 128, 16]
SOFF = [0, 128, 256, 384]

F32 = mybir.dt.float32
F16 = mybir.dt.float16
BF16 = mybir.dt.bfloat16
F8 = mybir.dt.float8e4
DR = mybir.MatmulPerfMode.DoubleRow

# fp8 pre-scales (host multiplies before the e4m3 cast; Exp un-scales)
SH = 16.0                 # hidden scale
SW = 64.0                 # W / w_copy scale
INV = 1.0 / (SH * SW)     # 1/1024

EXP_BUFS = 56   # in-flight exp tiles ([128,1024] f16)
OUT_BUFS = 2    # [128, 4096] bf16 output staging tiles (2 per block)


def _mm_splits(n):
    """Split a free-dim span into <=512 pieces aligned to 512 (PSUM banks)."""
    out = []
    off = 0
    while off < n:
        w = min(512, n - off)
        out.append((off, w))
        off += w
    return out


def build_program(with_bias: bool, b_copy: float, pad_corr: float):
    # Bacc (not plain Bass): its finalize() runs move_matmul_waits_to_ldweights
    # + generate_event_semaphores, which split multi-sem waits down to the
    # TRN2 limit of one wait per instruction — walrus rejects the IR otherwise.
    nc = bacc.Bacc()

    ebc = float(np.exp(b_copy))

    h8d = nc.dram_tensor("h8", [D, RH], F8, kind="ExternalInput")
    w8d = nc.dram_tensor("w8", [D, VSP], F8, kind="ExternalInput")
    h16d = nc.dram_tensor("h16", [D, LB * RB], F16, kind="ExternalInput")
    wc16d = nc.dram_tensor("wc16", [D, 1], F16, kind="ExternalInput")
    attnT = nc.dram_tensor("attnT", [S, LB * RB], F16, kind="ExternalInput")
    smap = nc.dram_tensor("smap", [LB, S, CV], F16, kind="ExternalInput")
    if with_bias:
        ebb = nc.dram_tensor("ebb", [128, VS], F32, kind="ExternalInput")

    og = nc.dram_tensor("og", [RH, VS], BF16, kind="ExternalOutput")
    oc = nc.dram_tensor("oc", [LB * RB, CV], F32, kind="ExternalOutput")

    with tile.TileContext(nc) as tc:
        with (
            tc.tile_pool(name="const", bufs=1) as const,
            tc.tile_pool(name="pm", bufs=2, space="PSUM") as pm,
            tc.tile_pool(name="pg", bufs=2, space="PSUM") as pg,
            tc.tile_pool(name="pc", bufs=1, space="PSUM") as pc,
            tc.tile_pool(name="expp", bufs=EXP_BUFS) as expp,
            tc.tile_pool(name="outp", bufs=OUT_BUFS) as outp,
            tc.tile_pool(name="ocp", bufs=2) as ocp,
            tc.tile_pool(name="smapp", bufs=4) as smapp,
            tc.tile_pool(name="small", bufs=10) as small,
            tc.tile_pool(name="gatep", bufs=NBL + LB) as gatep,
            tc.tile_pool(name="dram", bufs=1, space="DRAM") as dram,
        ):
            # ---------------- prologue ----------------
            # Dummy AllReduce first: even with the warmup NEFF, the FIRST
            # collective inside a NEFF pays ~12-18us of channel start plus
            # queueing; a junk 512B all-reduce with no data dependencies
            # absorbs that cost while the weights stream in, so the real
            # group-0 all-reduce (dispatched ~35us in) completes in ~1-3us.
            dum = small.tile([128, 1], F32, tag="dum", name="dum")
            nc.gpsimd.memset(dum[:], 0.0)
            dmi = dram.tile([128, 1], F32, tag="dmi", name="dmi")
            dmo = dram.tile([128, 1], F32, tag="dmo", name="dmo")
            nc.gpsimd.dma_start(dmi[:], dum[:])
            nc.gpsimd.collective_compute(
                "AllReduce",
                mybir.AluOpType.add,
                replica_groups=[[0, 1, 2, 3], [4, 5, 6, 7]],
                ins=[dmi.opt()],
                outs=[dmo.opt()],
            )
            # resident loads: fp8 tensors as [128, KC, free] k-plane tiles
            # (the middle dim is the DoubleRow k-pair axis).  hidden on the
            # gpsimd ring; the 4 MB W shard in 2048-col slices split across
            # the ACT and SP rings so block 0's chunks land within ~2-3us.
            h8t = const.tile([128, KC, RH], F8, tag="h8t", name="h8t")
            for k in range(KC):
                nc.gpsimd.dma_start(h8t[:, k:k + 1, :], h8d[k * 128:(k + 1) * 128, :])
            w8t = const.tile([128, KC, VSP], F8, tag="w8t", name="w8t")
            w_slices = [(0, 2048), (2048, 4096), (4096, 6144), (6144, VSP)]
            for (vo, ve) in w_slices:
                for k in range(KC):
                    # DMA-capable queues are SP/ACT/gpsimd only; split the
                    # 4 MB shard across the ACT and SP rings
                    eng = nc.scalar if k < 2 else nc.sync
                    eng.dma_start(
                        w8t[:, k:k + 1, vo:ve],
                        w8d[k * 128:(k + 1) * 128, vo:ve],
                    )
            # copy-branch inputs on the SP ring
            h16_t = []
            wc_t = []
            attnT_t = []
            ebb_t = []
            for k in range(KC):
                th = const.tile([128, LB * RB], F16, tag=f"h16_{k}", name=f"h16_{k}")
                nc.sync.dma_start(th[:], h16d[k * 128:(k + 1) * 128, :])
                h16_t.append(th)
                tw = const.tile([128, 1], F16, tag=f"wc16_{k}", name=f"wc16_{k}")
                nc.sync.dma_start(tw[:], wc16d[k * 128:(k + 1) * 128, :])
                wc_t.append(tw)
            for k in range(4):
                sk = SCH[k]
                t = const.tile([128, LB * RB], F16, tag=f"attnT{k}", name=f"attnT{k}")
                nc.sync.dma_start(t[:sk, :], attnT[SOFF[k]:SOFF[k] + sk, :])
                attnT_t.append(t)
            if with_bias:
                for i in range(NVC):
                    n = VCH[i] if i < 7 else RW7
                    t = const.tile([128, n], F32, tag=f"ebb{i}", name=f"ebb{i}")
                    nc.sync.dma_start(t[:], ebb[:, VOFF[i]:VOFF[i] + n])
                    ebb_t.append(t)

            # ---------------- main loop ----------------
            exp_tiles = [[None] * NVC for _ in range(NBL)]
            sg_tiles = [None] * NG    # group local sums [128, GROUP]
            cc_out = [None] * NG      # group all-reduced sums (SBUF)

            def compute_block(jb):
                cb = slice(jb * RB, (jb + 1) * RB)
                sp = small.tile([128, NVC], F32, tag="sp", name="sp")
                for i in range(NVC):
                    n = VCH[i]
                    rw = n if i < 7 else RW7
                    ps = pm.tile([128, 1024], F32, tag="pm", name="pm")
                    for kk in range(KC // 2):
                        for (o, w) in _mm_splits(n):
                            nc.tensor.matmul(
                                ps[:, o:o + w],
                                h8t[:, 2 * kk:2 * kk + 2, cb],
                                w8t[:, 2 * kk:2 * kk + 2,
                                    VOFF[i] + o:VOFF[i] + o + w],
                                start=(kk == 0), stop=(kk == KC // 2 - 1),
                                perf_mode=DR,
                            )
                    ex = expp.tile([128, 1024], F16, tag="exp", name="exp")
                    if with_bias:
                        nc.scalar.activation(
                            ex[:, :rw], ps[:, :rw],
                            mybir.ActivationFunctionType.Exp, scale=INV,
                        )
                        nc.vector.tensor_tensor(
                            ex[:, :rw], ex[:, :rw], ebb_t[i][:, :rw],
                            mybir.AluOpType.mult,
                        )
                        nc.vector.reduce_sum(
                            sp[:, i:i + 1], ex[:, :rw],
                            axis=mybir.AxisListType.X,
                        )
                    elif i in ACT_ACC:
                        nc.scalar.activation(
                            ex[:, :rw], ps[:, :rw],
                            mybir.ActivationFunctionType.Exp, scale=INV,
                            accum_out=sp[:, i:i + 1],
                        )
                    else:
                        nc.scalar.activation(
                            ex[:, :rw], ps[:, :rw],
                            mybir.ActivationFunctionType.Exp, scale=INV,
                        )
                        nc.vector.reduce_sum(
                            sp[:, i:i + 1], ex[:, :rw],
                            axis=mybir.AxisListType.X,
                        )
                    if i == 7:
                        # folded copy-gate numerator ep = exp(h . w_copy)
                        nc.scalar.activation(
                            ex[:, RW7:RW7 + 1], ps[:, RW7:RW7 + 1],
                            mybir.ActivationFunctionType.Exp, scale=INV,
                        )
                    exp_tiles[jb][i] = ex
                g, j = GRPOF[jb]
                nc.vector.reduce_sum(
                    sg_tiles[g][:, j:j + 1], sp[:], axis=mybir.AxisListType.X
                )

            def scale_block(jb):
                g, j = GRPOF[jb]
                sgl = cc_out[g]
                ept = exp_tiles[jb][7][:, RW7:RW7 + 1]
                # m = (1-gate)/S = 1 / ((1 + ep*e^{b_copy}) * (S - pad_corr))
                upl = small.tile([128, 1], F32, tag="upl", name="upl")
                if ebc == 1.0:
                    nc.vector.tensor_scalar_add(upl[:], ept, 1.0)
                else:
                    nc.vector.tensor_scalar(
                        upl[:], ept, ebc, 1.0,
                        mybir.AluOpType.mult, mybir.AluOpType.add,
                    )
                corr = small.tile([128, 1], F32, tag="corr", name="corr")
                nc.vector.tensor_scalar_add(corr[:], sgl[:, j:j + 1], -pad_corr)
                v = small.tile([128, 1], F32, tag="v", name="v")
                nc.vector.tensor_scalar(
                    v[:], corr[:], upl[:], None, mybir.AluOpType.mult
                )
                m = small.tile([128, 1], F32, tag="m", name="m")
                nc.vector.reciprocal(m[:], v[:])
                # scale exp chunks into bf16 staging tiles, 2 stores per block
                for half in range(2):
                    hn = 4096 if half == 0 else VS - 4096
                    ot = outp.tile([128, 4096], BF16, tag="ot", name="ot")
                    for i in range(4 * half, 4 * half + 4):
                        n = VCH[i] if i < 7 else RW7
                        oo = VOFF[i] - 4096 * half
                        nc.vector.tensor_scalar(
                            ot[:, oo:oo + n],
                            exp_tiles[jb][i][:, :n], m[:], None,
                            mybir.AluOpType.mult,
                        )
                        exp_tiles[jb][i] = None
                    nc.sync.dma_start(
                        og[jb * RB:(jb + 1) * RB, 4096 * half:4096 * half + hn],
                        ot[:, :hn],
                    )

            # ---------------- copy branch (batch-sharded) ----------------
            def emit_copy_branch():
                for l in range(LB):
                    tb = slice(l * RB, (l + 1) * RB)
                    # local gate: fp16 dot (sigma ~5e-4 -- the gate multiplies
                    # values that set the output absmax, so fp8 is too coarse)
                    gps = pg.tile([128, 1], F32, tag="gate", name="gate")
                    for k in range(KC):
                        nc.tensor.matmul(
                            gps[:], h16_t[k][:, tb], wc_t[k][:],
                            start=(k == 0), stop=(k == KC - 1),
                        )
                    el = gatep.tile([128, 1], F32, tag="el", name="el")
                    nc.scalar.activation(
                        el[:], gps[:], mybir.ActivationFunctionType.Exp,
                        bias=-float(b_copy), scale=-1.0,
                    )
                    ul = gatep.tile([128, 1], F32, tag="ul", name="ul")
                    nc.vector.tensor_scalar_add(ul[:], el[:], 1.0)
                    gl = gatep.tile([128, 1], F32, tag="gl", name="gl")
                    nc.vector.reciprocal(gl[:], ul[:])
                    cps = pc.tile([128, CV], F32, tag="cp", name="cp")
                    for k in range(4):
                        sk = SCH[k]
                        sm = smapp.tile([128, CV], F16, tag="sm", name="sm")
                        nc.sync.dma_start(
                            sm[:sk, :], smap[l, SOFF[k]:SOFF[k] + sk, :]
                        )
                        for (o, w) in _mm_splits(CV):
                            nc.tensor.matmul(
                                cps[:, o:o + w],
                                attnT_t[k][:sk, tb],
                                sm[:sk, o:o + w],
                                start=(k == 0), stop=(k == 3),
                            )
                    oct_ = ocp.tile([128, CV], F32, tag="oct", name="oct")
                    nc.vector.tensor_scalar(
                        oct_[:], cps[:], gl[:], None, mybir.AluOpType.mult
                    )
                    nc.sync.dma_start(oc[tb, :], oct_[:])

            for g in range(NG):
                gn = GS[g]
                sg_tiles[g] = small.tile([128, gn], F32, tag="sg", name="sg")
                for j in range(gn):
                    compute_block(GOFF[g] + j)
                # all-reduce this group's local sums across the 4 vocab shards
                cin = dram.tile([128, gn], F32, tag=f"cin{g}", name=f"cin{g}")
                cout = dram.tile([128, gn], F32, tag=f"cout{g}", name=f"cout{g}")
                nc.gpsimd.dma_start(cin[:], sg_tiles[g][:])
                nc.gpsimd.collective_compute(
                    "AllReduce",
                    mybir.AluOpType.add,
                    replica_groups=[[0, 1, 2, 3], [4, 5, 6, 7]],
                    ins=[cin.opt()],
                    outs=[cout.opt()],
                )
                sgl = small.tile([128, gn], F32, tag="sgl", name="sgl")
                nc.gpsimd.dma_start(sgl[:], cout[:])
                cc_out[g] = sgl
                if g == 0:
                    # PE-only work that needs neither w8 nor any collective:
                    # fills the PE while group 0's all-reduce is in flight
                    emit_copy_branch()
                for j in range(gn):
                    scale_block(GOFF[g] + j)


    nc.finalize()
    return nc


_warmed_up = False


def _warmup_collectives():
    """Run a minimal NEFF with one AllReduce so the collective channel
    (ncfw firmware / TOPSP) is warm before the main kernel executes —
    the first collective after boot costs ~60-75us of start latency."""
    global _warmed_up
    if _warmed_up:
        return
    nc = bacc.Bacc()
    x = nc.dram_tensor("x", [128, 4], F32, kind="ExternalInput")
    y = nc.dram_tensor("y", [128, 4], F32, kind="ExternalOutput")
    with tile.TileContext(nc) as tc:
        with (
            tc.tile_pool(name="sb", bufs=2) as sb,
            tc.tile_pool(name="dr", bufs=2, space="DRAM") as dr,
        ):
            t = sb.tile([128, 4], F32, tag="t", name="t")
            nc.sync.dma_start(t[:], x[:])
            bi = dr.tile([128, 4], F32, tag="bi", name="bi")
            bo = dr.tile([128, 4], F32, tag="bo", name="bo")
            nc.sync.dma_start(bi[:], t[:])
            nc.gpsimd.collective_compute(
                "AllReduce",
                mybir.AluOpType.add,
                replica_groups=[[0, 1, 2, 3], [4, 5, 6, 7]],
                ins=[bi.opt()],
                outs=[bo.opt()],
            )
            t2 = sb.tile([128, 4], F32, tag="t2", name="t2")
            nc.sync.dma_start(t2[:], bo[:])
            nc.sync.dma_start(y[:], t2[:])
    nc.finalize()
    z = np.zeros((128, 4), np.float32)
    run_bass_kernel_spmd(nc, [{"x": z}] * NCORES, core_ids=list(range(NCORES)))
    _warmed_up = True


def kernel(hidden, copy_attn, src_map, W, b, w_copy, b_copy, _trace=False):
    hidden = np.asarray(hidden, np.float32)
    copy_attn = np.asarray(copy_attn, np.float32)
    src_map = np.asarray(src_map, np.float32)
    W = np.asarray(W, np.float32)
    b = np.asarray(b, np.float32)
    w_copy = np.asarray(w_copy, np.float32)
    b_copy_f = float(np.asarray(b_copy))

    with_bias = bool(np.any(b != 0.0))
    pad_corr = float(np.exp(b[PAD])) if with_bias else 1.0

    # host-side shard prep (layout only; W[PAD,:] is dead data in the ref)
    Wz = W.copy()
    Wz[PAD, :] = 0.0
    W8 = (np.ascontiguousarray(Wz.T) * SW).astype(ml_dtypes.float8_e4m3)  # [D, V]
    wc8 = (w_copy.reshape(D, 1) * SW).astype(ml_dtypes.float8_e4m3)
    hT_f = np.ascontiguousarray(hidden.T)                            # [D, R] f32
    h8 = (hT_f * SH).astype(ml_dtypes.float8_e4m3)
    h16 = hT_f.astype(np.float16)
    wc16 = w_copy.reshape(D, 1).astype(np.float16)
    attnT_full = np.ascontiguousarray(copy_attn.T).astype(np.float16)  # [S, R]
    smap16 = src_map.astype(np.float16)                              # [B,S,CV]

    _warmup_collectives()
    nc = build_program(with_bias, b_copy_f, pad_corr)

    in_maps = []
    zpad = np.zeros((D, 15), dtype=ml_dtypes.float8_e4m3)
    for c in range(NCORES):
        h, q = divmod(c, NQ)
        rows = slice(h * RH, (h + 1) * RH)
        crows = slice(c * LB * RB, (c + 1) * LB * RB)
        m = {
            "h8": np.ascontiguousarray(h8[:, rows]),
            "w8": np.ascontiguousarray(
                np.concatenate(
                    [W8[:, q * VS:(q + 1) * VS], wc8, zpad], axis=1
                )
            ),
            "h16": np.ascontiguousarray(h16[:, crows]),
            "wc16": wc16,
            "attnT": np.ascontiguousarray(attnT_full[:, crows]),
            "smap": np.ascontiguousarray(smap16[c * LB:(c + 1) * LB]),
        }
        if with_bias:
            eb = np.exp(b[q * VS:(q + 1) * VS].astype(np.float64)).astype(
                np.float32
            )
            m["ebb"] = np.ascontiguousarray(
                np.broadcast_to(eb[None, :], (128, VS))
            )
        in_maps.append(m)

    trace_cores = None
    if os.environ.get("TRACE_ALL_CORES"):
        trace_cores = list(range(NCORES))
    res = run_bass_kernel_spmd(
        nc, in_maps, core_ids=list(range(NCORES)), trace=_trace,
        trace_cores=trace_cores,
    )

    out = np.empty((R, V + CV), np.float32)
    for c in range(NCORES):
        h, q = divmod(c, NQ)
        out[h * RH:(h + 1) * RH, q * VS:(q + 1) * VS] = (
            res.results[c]["og"].astype(np.float32)
        )
        out[c * LB * RB:(c + 1) * LB * RB, V:] = res.results[c]["oc"]
    out[:, PAD] = 0.0

    if _trace:
        kernel.last_results = res
    return out


kernel.last_results = None
